# revision 6
# baseline (speedup 1.0000x reference)
"""DigitCaps routing kernel for Trainium2 (8 NeuronCores, data parallel).

Math note: in the reference, `routers` starts at zero and is only ever
updated by adding a [B, 1] term broadcast along the feature axis, so each
row of `routers` is constant along features at every iteration.  Softmax of
a constant row is exactly uniform (exp(t - t) = 1 elementwise, sum = D), so
`agreements == 1/D` exactly in float32 at every routing iteration, and
`outputs` is identical across all 3 iterations.  The whole module therefore
reduces to a single pass:

    s[b] = (1/D) * sum_d x[b, d]          (D = 9216)
    v[b] = s * |s| / (1 + s^2)            (squash of a scalar)
    out  = norm over last dim of v.reshape(100, 10, 16)

which is a pure memory-bound row reduction over the 590 MB input.

Sharding: pure data parallel over batch — 2000 rows of 9216 floats per
core.  Each core computes its rows' squashed scalars v[2000]; the host
concatenates the 8 shards and does the final (16000-element) group norm.
"""

import numpy as np

import concourse.bass as bass
import concourse.bacc as bacc
import concourse.tile as tile
from concourse import mybir
from concourse.bass_utils import run_bass_kernel_spmd

N_CORES = 8
B = 16000
D = 9216  # 1152 * 8
ROWS = B // N_CORES  # 2000 rows per core
P = 128  # SBUF partitions
OUT_FEATURES = 10
OUT_CAPSULES = 16

# Row tiles per core: 15 full 128-row tiles + one 80-row tail.
TILES = [(i * P, min(P, ROWS - i * P)) for i in range((ROWS + P - 1) // P)]
NT = len(TILES)
N_FULL = sum(1 for _, r in TILES if r == P)  # 15


def build_nc(rows=ROWS, d=D, bufs=4):
    f32 = mybir.dt.float32
    tiles = [(i * P, min(P, rows - i * P)) for i in range((rows + P - 1) // P)]
    nt = len(tiles)
    n_full = sum(1 for _, r in tiles if r == P)

    nc = bacc.Bacc(None)
    x = nc.declare_dram_parameter("x", [rows, d], f32, isOutput=False)
    v = nc.declare_dram_parameter("v", [rows], f32, isOutput=True)

    with tile.TileContext(nc) as tc:
        with (
            tc.tile_pool(name="xpool", bufs=bufs) as xpool,
            tc.tile_pool(name="small", bufs=1) as small,
        ):
            sums = small.tile([P, nt], f32)
            for i, (r0, r) in enumerate(tiles):
                xt = xpool.tile([P, d], f32)
                nc.sync.dma_start(out=xt[:r, :], in_=x[r0 : r0 + r, :])
                nc.vector.reduce_sum(
                    out=sums[:r, i : i + 1], in_=xt[:r, :], axis=mybir.AxisListType.X
                )

            # squash: s = sums/d ; v = s * |s| / (1 + s^2), applied per
            # fully-populated slice of `sums` (the tail tile only fills the
            # first `r` partitions of its column, so it gets its own pass).
            s = small.tile([P, nt], f32)
            a = small.tile([P, nt], f32)
            denom = small.tile([P, nt], f32)
            rcp = small.tile([P, nt], f32)
            abss = small.tile([P, nt], f32)
            t = small.tile([P, nt], f32)
            vv = small.tile([P, nt], f32)

            def squash(sl):
                nc.scalar.mul(s[sl], sums[sl], 1.0 / d)
                nc.vector.tensor_mul(a[sl], s[sl], s[sl])
                nc.scalar.add(denom[sl], a[sl], 1.0)
                nc.vector.reciprocal(rcp[sl], denom[sl])
                nc.scalar.activation(
                    abss[sl], s[sl], func=mybir.ActivationFunctionType.Abs
                )
                nc.vector.tensor_mul(t[sl], abss[sl], rcp[sl])
                nc.vector.tensor_mul(vv[sl], s[sl], t[sl])

            if n_full:
                squash((slice(0, P), slice(0, n_full)))
            if n_full < nt:
                _, r = tiles[-1]
                squash((slice(0, r), slice(n_full, nt)))

            # v[n*128 + p] = vv[p, n] for the full tiles, then the tail.
            if n_full:
                nc.sync.dma_start(
                    out=v[0 : n_full * P].rearrange("(n p) -> p n", p=P),
                    in_=vv[:, 0:n_full],
                )
            if n_full < nt:
                r0, r = tiles[-1]
                nc.sync.dma_start(
                    out=v[r0 : r0 + r].rearrange("(p one) -> p one", one=1),
                    in_=vv[:r, n_full : n_full + 1],
                )
    return nc


_NC_CACHE = {}


def _get_nc():
    if "nc" not in _NC_CACHE:
        nc = build_nc()
        nc.finalize()  # runs Bacc legalization (wait splitting, reg alloc)
        _NC_CACHE["nc"] = nc
    return _NC_CACHE["nc"]


LAST_RESULTS = None  # BassKernelResults of the most recent run (for profiling)


def kernel(inputs: np.ndarray, *, _trace: bool = False, _trace_kwargs=None) -> np.ndarray:
    global LAST_RESULTS
    x = np.ascontiguousarray(np.asarray(inputs, dtype=np.float32)).reshape(B, D)
    in_maps = [{"x": x[c * ROWS : (c + 1) * ROWS]} for c in range(N_CORES)]
    nc = _get_nc()
    res = run_bass_kernel_spmd(
        nc,
        in_maps,
        core_ids=list(range(N_CORES)),
        trace=_trace,
        **(_trace_kwargs or {}),
    )
    LAST_RESULTS = res
    vfull = np.concatenate([np.asarray(res.results[c]["v"]) for c in range(N_CORES)])
    out = np.linalg.norm(
        vfull.reshape(-1, OUT_FEATURES, OUT_CAPSULES), axis=-1
    ).astype(np.float32)
    return out
